# revision 25
# baseline (speedup 1.0000x reference)
"""Trainium2 Bass kernel: nn_MultiHeadCrossAttention (B=4, S=1024, H=1024, 16 heads).

Sharding: 8 cores = (batch b in 0..3) x (head-group g in 0..1, 8 heads each).
Per core: q/k/v projections for its head group on its batch, flash-style
attention in scores-transposed layout (softmax along the PSUM partition axis
via an augmented ones-column in the v matmul), and a partial out-projection.
Host sums the two per-batch partials and adds the output bias.

All matmul operands are bf16 (PSUM accumulation stays fp32), which halves
input DMA and SBUF traffic; measured rel-l2 stays well under the 2e-2 gate.

Bias handling is exact and fully host-side:
  - bilinear span bias is constant along the softmax key axis -> cancels.
  - q bias: scores shift q.bk is constant along keys -> cancels; bq.k_j
    varies per key and is folded into the additive key bias `ebias`
    (computed on host: y_j @ (Wk.T @ bq) + bq.bk), together with the mask.
  - v/o bias: softmax rows sum to 1, so out picks up exactly Wo@bv + bo,
    added on host.

Softmax normalization never touches the PE and never blocks it: the ctx
halves are staged out of PSUM immediately (releasing the accumulators for
the next head pair), the two denominator rows are repartitioned to [128,8]
by small SBUF->SBUF DMAs for a lane-parallel exact reciprocal, and the
reciprocals are broadcast across partitions via a DRAM bounce (0-stride
partition read) before one full-width DVE multiply writes ctx.
"""
import os
import sys
import types

sys.path.insert(0, "/opt/trn_rl_repo")

# Optional NTFF profile hook shim (axon images lack antenv.axon_hooks).
if "antenv.axon_hooks" not in sys.modules:
    try:
        import trn_agent_boot.trn_boot as _tb

        _m = types.ModuleType("antenv.axon_hooks")
        _m.get_axon_ntff_profile_hook = (
            lambda: _tb._ntff_profile_via_ctypes("/opt/axon/libaxon_pjrt.so")
        )
        _m.set_axon_ntff_profile_hook = lambda h: None
        sys.modules["antenv.axon_hooks"] = _m
    except Exception:
        pass

import ml_dtypes
import numpy as np

import concourse.bass as bass
import concourse.mybir as mybir
import concourse.tile as tile
from concourse import bacc
from concourse.bass_utils import run_bass_kernel_spmd

F32 = mybir.dt.float32
F32R = mybir.dt.float32r
BF16 = mybir.dt.bfloat16
AF = mybir.ActivationFunctionType
BF16NP = ml_dtypes.bfloat16

B, S, H = 4, 1024, 1024
NHEADS = 16
HD = 64
G = 2                  # head groups (cores per batch)
NH = NHEADS // G       # 8 heads per core
F = NH * HD            # 512 per-core qkv features
HC = H // 128          # 8 contraction chunks for projections
KT = S // 128          # 8 key tiles
ST = S // 128          # 8 seq tiles
FC = F // 128          # 4 feature chunks (= head pairs)
NQ = S // 512          # 2 query halves
SCALE = float(HD) ** -0.5

# Augmented v region per head pair, width 160:
#   cols 0..63    v_even
#   col  64       1.0 (softmax denominator column, shared by both halves)
#   cols 65..95   0
#   cols 96..159  v_odd
# even head's ctx matmul uses cols [0:128]:  out p0-63=ctx_e, p64=sums_e
# odd  head's ctx matmul uses cols [32:160]: out p32=sums_o, p64-127=ctx_o
VREG = 160

_CACHE: dict = {}


def _build_nc():
    nc = bacc.Bacc("TRN2", target_bir_lowering=False, debug=False)

    xT = nc.dram_tensor("xT", [H, S], BF16, kind="ExternalInput")    # aspect[b].T
    yT = nc.dram_tensor("yT", [H, S], BF16, kind="ExternalInput")    # opinion[b].T
    wqT = nc.dram_tensor("wqT", [H, F], BF16, kind="ExternalInput")  # Wq[g].T
    wkT = nc.dram_tensor("wkT", [H, F], BF16, kind="ExternalInput")
    wvT = nc.dram_tensor("wvT", [H, F], BF16, kind="ExternalInput")
    woT = nc.dram_tensor("woT", [F, H], BF16, kind="ExternalInput")  # Wo[:, g].T
    ebias = nc.dram_tensor("ebias", [S], F32, kind="ExternalInput")  # per-key bias
    out = nc.dram_tensor("out", [S, H], BF16, kind="ExternalOutput")
    # DRAM bounce for the softmax reciprocals (DRAM APs allow the 0-stride
    # partition-broadcast read that SBUF APs reject).
    rsc = nc.dram_tensor("rsc", [NH // 2 * NQ, 2, 512], F32)

    with tile.TileContext(nc) as tc:
        const = tc.alloc_tile_pool(name="const", bufs=1)
        persist = tc.alloc_tile_pool(name="persist", bufs=1)
        psum = tc.alloc_tile_pool(name="psum", bufs=1, space="PSUM")

        eb_sb = const.tile([128, KT], F32, name="eb_sb")
        nc.sync.dma_start(out=eb_sb, in_=ebias.rearrange("(c p) -> p c", p=128))

        wv_sb = persist.tile([128, HC, F], BF16, name="wv_sb")
        wk_sb = persist.tile([128, HC, F], BF16, name="wk_sb")
        wq_sb = persist.tile([128, HC, F], BF16, name="wq_sb")
        yt_sb = persist.tile([128, HC, S], BF16, name="yt_sb")
        xt_sb = persist.tile([128, HC, S], BF16, name="xt_sb")
        wo_sb = persist.tile([128, FC, H], BF16, name="wo_sb")
        qT_sb = persist.tile([128, FC, S], BF16, name="qT_sb")
        kT_sb = persist.tile([128, FC, S], BF16, name="kT_sb")
        v_sb = persist.tile([128, KT, NH // 2, VREG], BF16, name="v_sb")
        ctx_sb = persist.tile([128, FC, S], BF16, name="ctx_sb")

        nc.vector.memset(
            v_sb.rearrange("p a b c -> p (a b) c")[:, :, 64:96], 0.0)
        nc.vector.memset(
            v_sb.rearrange("p a b c -> p (a b) c")[:, :, 64:65], 1.0)

        # ---- input DMA, first-need-first (all on the sync queue) ----------
        for hc in range(HC):
            nc.sync.dma_start(out=yt_sb[:, hc, 0:512],
                              in_=yT[hc * 128:(hc + 1) * 128, 0:512])
        for hc in range(HC):
            nc.sync.dma_start(out=wv_sb[:, hc, :],
                              in_=wvT[hc * 128:(hc + 1) * 128, :])
        for hc in range(HC):
            nc.sync.dma_start(out=yt_sb[:, hc, 512:1024],
                              in_=yT[hc * 128:(hc + 1) * 128, 512:1024])
        for hc in range(HC):
            nc.sync.dma_start(out=wk_sb[:, hc, :],
                              in_=wkT[hc * 128:(hc + 1) * 128, :])
        for hc in range(HC):
            nc.sync.dma_start(out=xt_sb[:, hc, :],
                              in_=xT[hc * 128:(hc + 1) * 128, :])
        for hc in range(HC):
            nc.sync.dma_start(out=wq_sb[:, hc, :],
                              in_=wqT[hc * 128:(hc + 1) * 128, :])
        for fc in range(FC):
            nc.sync.dma_start(out=wo_sb[:, fc, :],
                              in_=woT[fc * 128:(fc + 1) * 128, :])

        # ---- projection emitters ------------------------------------------
        def emit_vproj(st):
            ps = psum.tile([128, F], F32, name="vps", tag="pp", bufs=2)
            for hc in range(HC):
                nc.tensor.matmul(
                    ps,
                    yt_sb[:, hc, st * 128:(st + 1) * 128],
                    wv_sb[:, hc, :],
                    start=(hc == 0), stop=(hc == HC - 1),
                )
            pv = ps.rearrange("p (hp e d) -> p hp e d", hp=NH // 2, e=2)
            nc.vector.tensor_copy(v_sb[:, st, :, 0:64], pv[:, :, 0, :])
            nc.vector.tensor_copy(v_sb[:, st, :, 96:160], pv[:, :, 1, :])

        def emit_kqproj(which, fc, nq):
            src_sb, w_sb, dst_sb = (
                (yt_sb, wk_sb, kT_sb) if which == "k" else (xt_sb, wq_sb, qT_sb))
            ps = psum.tile([128, 512], F32, name="qkps", tag="pp", bufs=2)
            for hc in range(HC):
                nc.tensor.matmul(
                    ps,
                    w_sb[:, hc, fc * 128:(fc + 1) * 128],
                    src_sb[:, hc, nq * 512:(nq + 1) * 512],
                    start=(hc == 0), stop=(hc == HC - 1),
                )
            nc.vector.tensor_copy(dst_sb[:, fc, nq * 512:(nq + 1) * 512], ps)

        def emit_outproj(st, no):
            ps = psum.tile([128, 512], F32, name="ops", tag="pp", bufs=2)
            for fc2 in range(FC):
                nc.tensor.matmul(
                    ps,
                    ctx_sb[:, fc2, st * 128:(st + 1) * 128],
                    wo_sb[:, fc2, no * 512:(no + 1) * 512],
                    start=(fc2 == 0), stop=(fc2 == FC - 1),
                )
            ot = outsb.tile([128, 512], BF16, name="ot", tag="ot")
            nc.vector.tensor_copy(ot, ps)
            nc.sync.dma_start(
                out=out[st * 128:(st + 1) * 128, no * 512:(no + 1) * 512],
                in_=ot)

        # ---- attention block for one (nq, head pair) ----------------------
        def emit_attn(nq, hp):
            fc = hp
            cps_e = psum.tile([128, 512], F32, name="cps_e", tag="cps", bufs=2)
            cps_o = psum.tile([128, 512], F32, name="cps_o", tag="cps", bufs=2)
            exs = []
            for kt in range(KT):
                sps = psum.tile([128, 2, 512], F32, name="sps", tag="sps", bufs=2)
                for e in range(2):
                    p0 = 64 * e
                    # scoresT[k, q] = k_h . q_h over hd=64
                    nc.tensor.matmul(
                        sps[:, e, :],
                        kT_sb[p0:p0 + 64, fc, kt * 128:(kt + 1) * 128],
                        qT_sb[p0:p0 + 64, fc, nq * 512:(nq + 1) * 512],
                        start=True, stop=True,
                    )
                ex = exps.tile([128, 2, 512], BF16, name="ex", tag="ex")
                nc.scalar.activation(
                    ex, sps, AF.Exp,
                    bias=eb_sb[:, kt:kt + 1], scale=SCALE,
                )
                exs.append(ex)
            for kt in range(KT):
                nc.tensor.matmul(
                    cps_e,
                    v_sb[:, kt, hp, 0:128],
                    exs[kt][:, 0, :],
                    start=(kt == 0), stop=(kt == KT - 1),
                )
                nc.tensor.matmul(
                    cps_o,
                    v_sb[:, kt, hp, 32:160],
                    exs[kt][:, 1, :],
                    start=(kt == 0), stop=(kt == KT - 1),
                )
            # softmax denominators: sums_e at psum partition 64, sums_o at 32.
            # Exact DVE reciprocal in place, then broadcast across partitions
            # with two single-partition-contraction matmuls vs an all-ones
            # stationary (no DMA involved).
            # Stage the two ctx halves out of PSUM immediately so the cps
            # accumulators recycle without waiting for the normalization
            # chain (the next block's ctx matmuls reuse these banks).
            cc = smallp.tile([128, 512], F32, name="cc", tag="cc")
            nc.vector.tensor_copy(cc[0:64, :], cps_e[0:64, :])
            nc.vector.tensor_copy(cc[64:128, :], cps_o[64:128, :])
            # Single-partition rows are lane-serial on the DVE (~6.5 ns/elem),
            # so repartition the two 512-wide sums rows to [128,8] via small
            # SBUF->SBUF DMAs, run the lane-parallel exact reciprocal, and DMA
            # the reciprocals back into rows for the broadcast matmuls.
            srow = smallp.tile([128, 512], F32, name="srow", tag="srow")
            nc.vector.tensor_copy(srow[64:65, :], cps_e[64:65, :])
            nc.vector.tensor_copy(srow[32:33, :], cps_o[32:33, :])
            sp = smallp.tile([128, 8], F32, name="sp", tag="sp")
            nc.sync.dma_start(out=sp[:, 0:4], in_=srow[64:65, :])
            nc.sync.dma_start(out=sp[:, 4:8], in_=srow[32:33, :])
            rp = smallp.tile([128, 8], F32, name="rp", tag="rp")
            nc.vector.reciprocal(out=rp, in_=sp)
            it = fc * NQ + nq
            nc.gpsimd.dma_start(out=rsc[it, 0, :], in_=rp[:, 0:4])
            nc.gpsimd.dma_start(out=rsc[it, 1, :], in_=rp[:, 4:8])
            rbc = smallp.tile([128, 512], F32, name="rbc", tag="rbc")
            for e in range(2):
                srcap = rsc[it, e, :]
                nc.sync.dma_start(
                    out=rbc[64 * e:64 * e + 64, :],
                    in_=bass.AP(tensor=srcap.tensor, offset=srcap.offset,
                                ap=[[0, 64]] + list(srcap.ap)))
            nc.vector.tensor_mul(
                ctx_sb[:, fc, nq * 512:(nq + 1) * 512], cc, rbc)

        # ---- emission schedule --------------------------------------------
        with tc.tile_pool(name="exps", bufs=4) as exps, \
             tc.tile_pool(name="outsb", bufs=3) as outsb, \
             tc.tile_pool(name="smallp", bufs=2) as smallp:

            for st in range(ST):
                emit_vproj(st)
            for nq in range(NQ):
                emit_kqproj("k", 0, nq)
            for nq in range(NQ):
                emit_kqproj("q", 0, nq)

            for nq in range(NQ):
                for hp in range(NH // 2):
                    emit_attn(nq, hp)
                    if nq == 0 and hp < NH // 2 - 1:
                        for n2 in range(NQ):
                            emit_kqproj("k", hp + 1, n2)
                        for n2 in range(NQ):
                            emit_kqproj("q", hp + 1, n2)
                for st in range(4 * nq, 4 * nq + 4):
                    for no in range(NQ):
                        emit_outproj(st, no)

        psum.release()
        persist.release()
        const.release()

    nc.finalize()
    return nc


def get_nc():
    if "nc" not in _CACHE:
        _CACHE["nc"] = _build_nc()
    return _CACHE["nc"]


def make_in_maps(aspect_hidden, opinion_hidden, attention_mask,
                 Wq, bq, Wk, bk, Wv, bv, Wo, bo):
    asp = np.asarray(aspect_hidden, np.float32)
    opi = np.asarray(opinion_hidden, np.float32)
    mask = np.asarray(attention_mask)
    xTs = [np.ascontiguousarray(asp[b].T.astype(BF16NP)) for b in range(B)]
    yTs = [np.ascontiguousarray(opi[b].T.astype(BF16NP)) for b in range(B)]
    # per-key score bias: mask plus the non-cancelling q-bias term
    # bq . k_j = y_j @ (Wk.T @ bq) + bq . bk  (exact, fp64 on host)
    u = Wk.astype(np.float64).T @ bq.astype(np.float64)
    qk_const = float(bq.astype(np.float64) @ bk.astype(np.float64))
    ebs = []
    for b in range(B):
        # activation computes exp(scores_raw*SCALE + ebias), so the key bias
        # enters pre-scaled; masked keys get a huge negative (exp -> 0).
        eb = SCALE * (opi[b].astype(np.float64) @ u + qk_const)
        eb = np.where(mask[b] == 0, -1e30, eb)
        ebs.append(eb.astype(np.float32))
    wqTs = [np.ascontiguousarray(Wq[g * F:(g + 1) * F, :].T.astype(BF16NP))
            for g in range(G)]
    wkTs = [np.ascontiguousarray(Wk[g * F:(g + 1) * F, :].T.astype(BF16NP))
            for g in range(G)]
    wvTs = [np.ascontiguousarray(Wv[g * F:(g + 1) * F, :].T.astype(BF16NP))
            for g in range(G)]
    woTs = [np.ascontiguousarray(Wo[:, g * F:(g + 1) * F].T.astype(BF16NP))
            for g in range(G)]
    in_maps = []
    for c in range(8):
        b, g = c // G, c % G
        in_maps.append({
            "xT": xTs[b], "yT": yTs[b],
            "wqT": wqTs[g], "wkT": wkTs[g], "wvT": wvTs[g], "woT": woTs[g],
            "ebias": ebs[b],
        })
    return in_maps


def kernel(aspect_hidden, opinion_hidden, attention_mask,
           Wq, bq, Wk, bk, Wv, bv, Wo, bo, Wbil, bbil):
    Wq = np.asarray(Wq, np.float32); bq = np.asarray(bq, np.float32)
    Wk = np.asarray(Wk, np.float32); bk = np.asarray(bk, np.float32)
    Wv = np.asarray(Wv, np.float32); bv = np.asarray(bv, np.float32)
    Wo = np.asarray(Wo, np.float32); bo = np.asarray(bo, np.float32)

    nc = get_nc()
    in_maps = make_in_maps(aspect_hidden, opinion_hidden, attention_mask,
                           Wq, bq, Wk, bk, Wv, bv, Wo, bo)
    trace = bool(int(os.environ.get("KERNEL_TRACE", "0")))
    res = run_bass_kernel_spmd(nc, in_maps, core_ids=list(range(8)), trace=trace)
    _CACHE["last_results"] = res

    # v-bias folds into a constant output offset: softmax rows sum to 1, so
    # ctx picks up +bv exactly, and out picks up +Wo @ bv.
    bo_eff = (bo.astype(np.float64) + Wo.astype(np.float64) @ bv.astype(np.float64))
    outs = np.empty((B, S, H), np.float32)
    for b in range(B):
        acc = (res.results[G * b]["out"].astype(np.float64)
               + res.results[G * b + 1]["out"].astype(np.float64) + bo_eff)
        outs[b] = acc.astype(np.float32)
    return outs
